# revision 11
# baseline (speedup 1.0000x reference)
"""Slot-attention kernel for Trainium2, SPMD over 8 NeuronCores (v2).

Reference computation (per batch element b):
  query[b,n,:] = q[n,b,:] @ qw[n]          (n = 32 query slots)
  keyp [b,m,:] = k[m,b,:] @ kw[m]          (m = 32 key slots)
  value[b,m,:] = k[m,b,:] @ vw[m]
  logits[b,n,m] = query[b,n,:]·keyp[b,m,:] / 16
  attn = softmax_m(logits)
  out[n,b,:] = sum_m attn[b,n,m] * value[b,m,:]

Sharding: data-parallel over batch (4096 -> 512 per core), weights replicated.

v2 changes vs v1:
  - host pre-packs q/k/weights so every DMA lands >=2KB contiguous per
    partition (v1 had 512B descriptors, DMA engines were 60% busy).
  - q/k projection weights are loaded once and stay SBUF-resident across
    both batch halves (v1 re-DMAed all weights per half: -8.4MB HBM).
  - output is written bf16 (host upcasts) through a single 128-partition
    DMA per 8-batch group (v1: f32 via 4x 32-partition DMAs).
  - psum evacuations fused: Q proj t0+t1 in one [128,512] op, 1/16
    temperature split as 1/4 on Q and 1/4 on K.
  - E->attn^T transposes merged 4 groups per DVE op.
  - phase B interleaved per 2 quads so PE never idles long (HAM stays warm).
"""

import numpy as np
import ml_dtypes

import concourse.bass as bass
from concourse import bacc
import concourse.mybir as mybir
import concourse.tile as tile
from concourse.bass_utils import run_bass_kernel_spmd
import concourse.bass_utils as _bu

# walrus defaults to --enable-ldw-opt=false, which forces every matmul to
# serialize behind its weight load (no background weight buffer, exposed
# drain between matmuls). Enable it for OUR kernel only: the flag must not
# leak into the framework's glue NEFFs (their transpose-mode Ldweights is
# genuinely incompatible and fails codegen).
if not getattr(_bu, "_ldw_opt_patched", False):
    _orig_run_command = _bu.run_command

    def _run_command_ldw(cmd, **kw):
        if any(str(c).endswith("file.neff") for c in cmd):
            cmd = [("--enable-ldw-opt=true" if c == "--enable-ldw-opt=false"
                    else c) for c in cmd]
        return _orig_run_command(cmd, **kw)

    _bu.run_command = _run_command_ldw
    _bu._ldw_opt_patched = True

BF16 = mybir.dt.bfloat16
F32 = mybir.dt.float32

NQ = 32          # query slots
NK = 32          # key slots
D = 256          # input dim (contraction of projections)
A = 256          # attn dim (contraction of logits)
O = 256          # out dim
BS = 4096
N_CORES = 8
BS_CORE = BS // N_CORES   # 512
N_HALVES = 2
B_H = BS_CORE // N_HALVES  # 256
N_GROUPS = B_H // 4        # 64 groups of 4 batches per half
N_QUADS = N_GROUPS // 4    # 16


def build_kernel():
    nc = bacc.Bacc()

    # host-packed inputs; every DMA slice is contiguous per partition
    xH = nc.declare_dram_parameter(
        "xH", [N_HALVES, 16, 128, 2, 2, 2, B_H], BF16, isOutput=False
    )  # [half, sg, p, qk, s, c, b]
    wqk = nc.declare_dram_parameter(
        "wqk", [128, NQ, 2, 2, A], BF16, isOutput=False
    )  # [p, slot, c, qk, a]
    wvH = nc.declare_dram_parameter(
        "wvH", [16, 128, 2, 2, O], BF16, isOutput=False
    )  # [sg, p, s, c, o]
    # batch = 256*half + 64*j + g; host un-permutes to [n, b, o]
    out = nc.declare_dram_parameter(
        "out", [N_HALVES, 4, NQ, N_GROUPS, O], BF16, isOutput=True)
    out_r = out.rearrange("h j n g o -> h (j n) g o")

    with tile.TileContext(nc) as tc:
        with (
            tc.tile_pool(name="wpool", bufs=1) as wpool,
            tc.tile_pool(name="xin", bufs=4) as xin,
            tc.tile_pool(name="wvin", bufs=2) as wvin,
            tc.tile_pool(name="big", bufs=1) as big,
            tc.tile_pool(name="vn", bufs=8) as vn,
            tc.tile_pool(name="etp", bufs=4) as etp,
            tc.tile_pool(name="smp", bufs=4) as smp,
            tc.tile_pool(name="outp", bufs=2) as outp,
            # shared pool: proj evac tiles in phase A, attn@V in phase B
            tc.tile_pool(name="mm_ps", bufs=6, space="PSUM") as mm_ps,
            tc.tile_pool(name="lg_ps", bufs=2, space="PSUM") as lg_ps,
        ):
            # resident q/k projection weights: 64KB/partition, loaded once.
            # weight loads ride the ACT HWDGE ring so the SP ring carries
            # only the per-sg activation stream (no head-of-line blocking).
            # chunks 1..7 are interleaved into the first sg iterations so
            # wvs(sg0) is not stuck behind 8.4MB of projection weights.
            wq = wpool.tile([128, NQ, 2, 2, A], BF16, tag="wq")

            def load_wq_chunk(ch):
                nc.scalar.dma_start(
                    out=wq[:, 4 * ch:4 * (ch + 1)],
                    in_=wqk[:, 4 * ch:4 * (ch + 1)],
                )

            for half in range(N_HALVES):
                # ---- Phase A: projections ----
                QTs = big.tile([128, 2, NQ, B_H], BF16, tag="QTs")  # [p,t,n,b]
                KTs = big.tile([128, 2, NK, B_H], BF16, tag="KTs")
                # V32Q[32j+m, g, o] = value[b=64j+g][m, o]
                V32Q = big.tile([128, N_GROUPS, O], BF16, tag="V32Q")

                # partition view for 2-row shuffle writes:
                # p = 64*bc + 32*jj + m  ->  [bc, jj, m]
                V32Q_r = V32Q.rearrange("(bc q) g o -> bc q g o", bc=2)

                for sg in range(16):
                    xts = xin.tile([128, 2, 2, 2, B_H], BF16, tag="xts")
                    nc.sync.dma_start(out=xts, in_=xH[half, sg])
                    wvs = wvin.tile([128, 2, 2, O], BF16, tag="wvs")
                    nc.scalar.dma_start(out=wvs, in_=wvH[sg])

                    # V projections first so the V32Q scatter DMAs start
                    # draining while the Q/K projections still run
                    for si in range(2):
                        s = 2 * sg + si
                        # V projection -> [b, o] (stationary = k activations)
                        psv = mm_ps.tile([128, 2, O], F32, tag="ps")
                        for bc in range(2):
                            for c in range(2):
                                nc.tensor.matmul(
                                    psv[:, bc, :],
                                    lhsT=xts[:, 1, si, c,
                                             128 * bc:128 * (bc + 1)],
                                    rhs=wvs[:, si, c, :],
                                    start=(c == 0),
                                    stop=(c == 1),
                                )
                        VN = vn.tile([128, 2, O], BF16, tag="VN")
                        if s % 2 == 0:
                            nc.vector.tensor_copy(out=VN, in_=psv)
                        else:
                            nc.scalar.copy(out=VN, in_=psv)
                        # scatter rows {64bc+s, 64bc+32+s} <- VN[:, bc, :]
                        # (dest iterates (jj, g, o), src (p=64jj+g, o): match)
                        for bc in range(2):
                            nc.gpsimd.dma_start(
                                out=V32Q_r[bc, s::32, :, :],
                                in_=VN[:, bc, :],
                            )
                    for si in range(2):
                        s = 2 * sg + si
                        # Q and K projections -> [a, b]; 1/4 scale on each
                        # so logits carry the full 1/16 temperature
                        for w in range(2):
                            ps = mm_ps.tile([128, 2, B_H], F32, tag="ps")
                            for t in range(2):
                                for c in range(2):
                                    nc.tensor.matmul(
                                        ps[:, t, :],
                                        lhsT=wq[:, s, c, w,
                                                128 * t:128 * (t + 1)],
                                        rhs=xts[:, w, si, c, :],
                                        start=(c == 0),
                                        stop=(c == 1),
                                    )
                            dst = QTs if w == 0 else KTs
                            if (s + w) % 2 == 0:
                                nc.scalar.mul(dst[:, :, s, :], ps, 0.25)
                            else:
                                nc.vector.tensor_scalar_mul(
                                    out=dst[:, :, s, :], in0=ps, scalar1=0.25)

                # ---- Phase B: logits, softmax, attn @ V, store ----
                E = big.tile([128, N_GROUPS, NK], BF16, tag="E")
                rs = big.tile([128, N_GROUPS], F32, tag="rs")

                def do_quad(gq):
                    lg = lg_ps.tile([128, 4, NK], F32, tag="lg")
                    for qi in range(4):
                        g = 4 * gq + qi
                        for c in range(2):
                            for j in range(4):
                                b = 64 * j + g
                                nc.tensor.matmul(
                                    lg[32 * j:32 * (j + 1), qi, :],
                                    lhsT=QTs[:, c, :, b],
                                    rhs=KTs[:, c, :, b],
                                    start=(c == 0),
                                    stop=(c == 1),
                                    tile_position=(0, 32 * j),
                                    skip_group_check=True,
                                )
                    # softmax without max-subtraction: |logits| <= ~2.5
                    nc.scalar.activation(
                        out=E[:, 4 * gq:4 * gq + 4, :].rearrange(
                            "p a b -> p (a b)"),
                        in_=lg.rearrange("p a b -> p (a b)"),
                        func=mybir.ActivationFunctionType.Exp,
                    )

                for chunk in range(8):   # 8 groups per chunk
                    g0 = 8 * chunk
                    do_quad(2 * chunk)
                    do_quad(2 * chunk + 1)
                    sm = smp.tile([128, 8], F32, tag="sm")
                    nc.vector.reduce_sum(
                        out=sm, in_=E[:, g0:g0 + 8, :],
                        axis=mybir.AxisListType.X,
                    )
                    nc.vector.reciprocal(out=rs[:, g0:g0 + 8], in_=sm)
                    OUTo = outp.tile([128, 8, O], BF16, tag="OUTo")
                    for gg in (0, 4):
                        # 4-group attn transpose: te4[32j+m, g', n]
                        te4 = etp.tile([128, 4, NK], BF16, tag="te4")
                        nc.vector.transpose(
                            out=te4.rearrange("p a b -> p (a b)"),
                            in_=E[:, g0 + gg:g0 + gg + 4, :].rearrange(
                                "p a b -> p (a b)"),
                        )
                        for g2 in range(4):
                            g = g0 + gg + g2
                            av = mm_ps.tile([128, O], F32, tag="ps")
                            for j in range(4):
                                nc.tensor.matmul(
                                    av[32 * j:32 * (j + 1), :],
                                    lhsT=te4[32 * j:32 * (j + 1), g2, :],
                                    rhs=V32Q[32 * j:32 * (j + 1), g, :],
                                    start=True, stop=True,
                                    tile_position=(32 * j, 32 * j),
                                    skip_group_check=True,
                                )
                            # psum -> sbuf with 1/softmax-sum row scaling
                            if g % 2 == 0:
                                nc.scalar.mul(OUTo[:, g - g0, :], av,
                                              rs[:, g:g + 1])
                            else:
                                nc.vector.tensor_scalar_mul(
                                    out=OUTo[:, g - g0, :], in0=av,
                                    scalar1=rs[:, g:g + 1])
                    nc.sync.dma_start(
                        out=out_r[half, :, g0:g0 + 8, :], in_=OUTo)
    return nc


def _prep_inputs(q, k, query_weight, key_weight, value_weight):
    bf = ml_dtypes.bfloat16
    q = np.asarray(q, dtype=np.float32).astype(bf)
    k = np.asarray(k, dtype=np.float32).astype(bf)
    # xH[ci, half, sg, p, qk, s, c, b] = {q,k}[2sg+s, 512ci+256h+b, 128c+p]
    def pack_x(x):
        t = x.reshape(16, 2, N_CORES, N_HALVES, B_H, 2, 128)
        return t.transpose(2, 3, 0, 6, 1, 5, 4)  # [ci,half,sg,p,s,c,b]
    tq = pack_x(q)
    tk = pack_x(k)
    xAll = np.ascontiguousarray(
        np.stack((tq, tk), axis=4))  # [ci,half,sg,p,qk,s,c,b]
    # wqk[p, slot, c, qk, a]
    ws = np.stack(
        (np.asarray(query_weight, np.float32),
         np.asarray(key_weight, np.float32)), axis=2)  # [n, d, qk, a]
    wqk = np.ascontiguousarray(
        ws.reshape(NQ, 2, 128, 2, A).transpose(2, 0, 1, 3, 4).astype(bf))
    # wvH[sg, p, s, c, o]
    wv = np.ascontiguousarray(
        np.asarray(value_weight, np.float32)
        .reshape(16, 2, 2, 128, O).transpose(0, 3, 1, 2, 4).astype(bf))
    in_maps = []
    for i in range(N_CORES):
        in_maps.append({"xH": np.ascontiguousarray(xAll[i]),
                        "wqk": wqk, "wvH": wv})
    return in_maps


_NC_CACHE = {}


def _get_nc():
    if "nc" not in _NC_CACHE:
        nc = build_kernel()
        nc.finalize()
        _NC_CACHE["nc"] = nc
    return _NC_CACHE["nc"]


def kernel(q, k, query_weight, key_weight, value_weight, _trace=False):
    nc = _get_nc()
    in_maps = _prep_inputs(q, k, query_weight, key_weight, value_weight)
    res = run_bass_kernel_spmd(nc, in_maps, core_ids=list(range(N_CORES)),
                               trace=_trace)
    outs = []
    for i in range(N_CORES):
        o = res.results[i]["out"]  # [half, j, n, g, o]
        outs.append(o.transpose(2, 0, 1, 3, 4).reshape(NQ, BS_CORE, O))
    full = np.concatenate(outs, axis=1).astype(np.float32)
    if _trace:
        return full, res
    return full


# revision 14
# speedup vs baseline: 1.0088x; 1.0088x over previous
"""Slot-attention kernel for Trainium2, SPMD over 8 NeuronCores (v2).

Reference computation (per batch element b):
  query[b,n,:] = q[n,b,:] @ qw[n]          (n = 32 query slots)
  keyp [b,m,:] = k[m,b,:] @ kw[m]          (m = 32 key slots)
  value[b,m,:] = k[m,b,:] @ vw[m]
  logits[b,n,m] = query[b,n,:]·keyp[b,m,:] / 16
  attn = softmax_m(logits)
  out[n,b,:] = sum_m attn[b,n,m] * value[b,m,:]

Sharding: data-parallel over batch (4096 -> 512 per core), weights replicated.

v2 changes vs v1:
  - host pre-packs q/k/weights so every DMA lands >=2KB contiguous per
    partition (v1 had 512B descriptors, DMA engines were 60% busy).
  - q/k projection weights are loaded once and stay SBUF-resident across
    both batch halves (v1 re-DMAed all weights per half: -8.4MB HBM).
  - output is written bf16 (host upcasts) through a single 128-partition
    DMA per 8-batch group (v1: f32 via 4x 32-partition DMAs).
  - psum evacuations fused: Q proj t0+t1 in one [128,512] op, 1/16
    temperature split as 1/4 on Q and 1/4 on K.
  - E->attn^T transposes merged 4 groups per DVE op.
  - phase B interleaved per 2 quads so PE never idles long (HAM stays warm).
"""

import numpy as np
import ml_dtypes

import concourse.bass as bass
from concourse import bacc
import concourse.mybir as mybir
import concourse.tile as tile
from concourse.bass_utils import run_bass_kernel_spmd
import concourse.bass_utils as _bu

# walrus defaults to --enable-ldw-opt=false, which forces every matmul to
# serialize behind its weight load (no background weight buffer, exposed
# ~220-cycle drain between matmuls). walrus rejects ldw-opt whenever an
# Ldweights instruction carries semaphores, and the Tile scheduler puts the
# weight-tile DMA wait on the Ldweights of each pair. Fix: rewrite the BIR
# before compiling — hoist each Ldweights' on_wait onto an inserted
# wait-only EventSemaphore (same engine, immediately before; in-order queue
# makes this semantically identical) — then compile THIS kernel's NEFF with
# --enable-ldw-opt=true. The flag must not leak into other NEFFs compiled
# through the same hook.
import json as _json

_IN_KERNEL_COMPILE = [False]


def _rewrite_bir_for_ldw_opt(bir_json):
    """Fold each standalone Ldweights into its Matmult (ldweights=true).

    walrus's LDW optimization rejects standalone InstLdweights outright;
    self-loading Matmults are the shape it knows how to hoist. The weight AP
    is already ins[1] of the Matmult. A self-loading Matmult's LW encoding
    holds at most ONE sync wait, so surplus waits (the Ldweights' wait plus
    the Matmult's own) spill onto wait-only EventSemaphores placed just
    before it — semantically identical on the in-order engine queue.
    """
    d = _json.loads(bir_json)
    n_es = 0
    for fn in d.get("functions", []):
        for blk in fn.get("blocks", []):
            insts = blk.get("instructions")
            if not insts:
                continue
            out = []
            pending = {}
            for inst in insts:
                op = inst.get("opcode")
                eng = inst.get("engine")
                if op == "Ldweights":
                    assert eng not in pending, "unpaired Ldweights"
                    pending[eng] = inst
                    continue
                if op == "Matmult" and eng in pending:
                    ldw = pending.pop(eng)
                    inst["ldweights"] = True
                    lw = (ldw.get("sync_info") or {}).get("on_wait") or []
                    msi = inst.get("sync_info") or {"on_wait": [],
                                                    "on_update": []}
                    waits = list(lw) + list(msi.get("on_wait") or [])
                    if len(waits) <= 1:
                        msi["on_wait"] = waits
                    else:
                        for w in waits[:-1]:
                            out.append({
                                "debug": inst.get("debug", 0), "engine": eng,
                                "ins": [], "outs": [],
                                "name": f"I-ldwfix-{n_es}",
                                "opcode": "EventSemaphore",
                                "sync_info": {"on_update": [],
                                              "on_wait": [w]},
                            })
                            n_es += 1
                        msi["on_wait"] = [waits[-1]]
                    inst["sync_info"] = msi
                out.append(inst)
            assert not pending, "Ldweights without following Matmult"
            blk["instructions"] = out
    return _json.dumps(d, separators=(",", ":")).encode()


if not getattr(_bu, "_ldw_opt_patched", False):
    _orig_run_command = _bu.run_command

    def _run_command_ldw(cmd, **kw):
        if _IN_KERNEL_COMPILE[0]:
            cmd = [("--enable-ldw-opt=true" if c == "--enable-ldw-opt=false"
                    else c) for c in cmd]
        return _orig_run_command(cmd, **kw)

    _bu.run_command = _run_command_ldw

    _orig_compile_bir = _bu.compile_bir_kernel

    def _compile_bir_ldw(bir_json, tmpdir, neff_name="file.neff"):
        raw = bir_json if isinstance(bir_json, bytes) else bir_json.encode()
        if b'"wqk"' in raw:
            bir_json = _rewrite_bir_for_ldw_opt(raw)
            _IN_KERNEL_COMPILE[0] = True
            try:
                return _orig_compile_bir(bir_json, tmpdir, neff_name)
            finally:
                _IN_KERNEL_COMPILE[0] = False
        return _orig_compile_bir(bir_json, tmpdir, neff_name)

    _bu.compile_bir_kernel = _compile_bir_ldw
    from concourse import bass2jax as _b2j
    _b2j.compile_bir_kernel = _compile_bir_ldw
    _bu._ldw_opt_patched = True

BF16 = mybir.dt.bfloat16
F32 = mybir.dt.float32

NQ = 32          # query slots
NK = 32          # key slots
D = 256          # input dim (contraction of projections)
A = 256          # attn dim (contraction of logits)
O = 256          # out dim
BS = 4096
N_CORES = 8
BS_CORE = BS // N_CORES   # 512
N_HALVES = 2
B_H = BS_CORE // N_HALVES  # 256
N_GROUPS = B_H // 4        # 64 groups of 4 batches per half
N_QUADS = N_GROUPS // 4    # 16


def build_kernel():
    nc = bacc.Bacc()

    # host-packed inputs; every DMA slice is contiguous per partition
    xH = nc.declare_dram_parameter(
        "xH", [N_HALVES, 16, 128, 2, 2, 2, B_H], BF16, isOutput=False
    )  # [half, sg, p, qk, s, c, b]
    wqk = nc.declare_dram_parameter(
        "wqk", [128, NQ, 2, 2, A], BF16, isOutput=False
    )  # [p, slot, c, qk, a]
    wvH = nc.declare_dram_parameter(
        "wvH", [16, 128, 2, 2, O], BF16, isOutput=False
    )  # [sg, p, s, c, o]
    # batch = 256*half + 64*j + g; host un-permutes to [n, b, o]
    out = nc.declare_dram_parameter(
        "out", [N_HALVES, 4, NQ, N_GROUPS, O], BF16, isOutput=True)
    out_r = out.rearrange("h j n g o -> h (j n) g o")

    with tile.TileContext(nc) as tc:
        with (
            tc.tile_pool(name="wpool", bufs=1) as wpool,
            tc.tile_pool(name="xin", bufs=4) as xin,
            tc.tile_pool(name="wvin", bufs=2) as wvin,
            tc.tile_pool(name="big", bufs=1) as big,
            tc.tile_pool(name="vn", bufs=8) as vn,
            tc.tile_pool(name="etp", bufs=4) as etp,
            tc.tile_pool(name="smp", bufs=4) as smp,
            tc.tile_pool(name="outp", bufs=2) as outp,
            # shared pool: proj evac tiles in phase A, attn@V in phase B
            tc.tile_pool(name="mm_ps", bufs=6, space="PSUM") as mm_ps,
            tc.tile_pool(name="lg_ps", bufs=2, space="PSUM") as lg_ps,
        ):
            # resident q/k projection weights: 64KB/partition, loaded once.
            # weight loads ride the ACT HWDGE ring so the SP ring carries
            # only the per-sg activation stream (no head-of-line blocking).
            # chunks 1..7 are interleaved into the first sg iterations so
            # wvs(sg0) is not stuck behind 8.4MB of projection weights.
            wq = wpool.tile([128, NQ, 2, 2, A], BF16, tag="wq")

            def load_wq_chunk(ch):
                nc.scalar.dma_start(
                    out=wq[:, 4 * ch:4 * (ch + 1)],
                    in_=wqk[:, 4 * ch:4 * (ch + 1)],
                )

            for half in range(N_HALVES):
                # ---- Phase A: projections ----
                QTs = big.tile([128, 2, NQ, B_H], BF16, tag="QTs")  # [p,t,n,b]
                KTs = big.tile([128, 2, NK, B_H], BF16, tag="KTs")
                # V32Q[32j+m, g, o] = value[b=64j+g][m, o]
                V32Q = big.tile([128, N_GROUPS, O], BF16, tag="V32Q")

                # partition view for 2-row shuffle writes:
                # p = 64*bc + 32*jj + m  ->  [bc, jj, m]
                V32Q_r = V32Q.rearrange("(bc q) g o -> bc q g o", bc=2)

                for sg in range(16):
                    xts = xin.tile([128, 2, 2, 2, B_H], BF16, tag="xts")
                    nc.sync.dma_start(out=xts, in_=xH[half, sg])
                    wvs = wvin.tile([128, 2, 2, O], BF16, tag="wvs")
                    nc.scalar.dma_start(out=wvs, in_=wvH[sg])
                    if half == 0:
                        if sg == 0:
                            load_wq_chunk(0)
                        elif sg <= 7:
                            load_wq_chunk(sg)

                    # V projections first so the V32Q scatter DMAs start
                    # draining while the Q/K projections still run
                    for si in range(2):
                        s = 2 * sg + si
                        # V projection -> [b, o] (stationary = k activations)
                        psv = mm_ps.tile([128, 2, O], F32, tag="ps")
                        for bc in range(2):
                            for c in range(2):
                                nc.tensor.matmul(
                                    psv[:, bc, :],
                                    lhsT=xts[:, 1, si, c,
                                             128 * bc:128 * (bc + 1)],
                                    rhs=wvs[:, si, c, :],
                                    start=(c == 0),
                                    stop=(c == 1),
                                )
                        VN = vn.tile([128, 2, O], BF16, tag="VN")
                        if s % 2 == 0:
                            nc.vector.tensor_copy(out=VN, in_=psv)
                        else:
                            nc.scalar.copy(out=VN, in_=psv)
                        # scatter rows {64bc+s, 64bc+32+s} <- VN[:, bc, :]
                        # (dest iterates (jj, g, o), src (p=64jj+g, o): match)
                        for bc in range(2):
                            nc.gpsimd.dma_start(
                                out=V32Q_r[bc, s::32, :, :],
                                in_=VN[:, bc, :],
                            )
                    for si in range(2):
                        s = 2 * sg + si
                        # Q and K projections -> [a, b]; 1/4 scale on each
                        # so logits carry the full 1/16 temperature
                        for w in range(2):
                            ps = mm_ps.tile([128, 2, B_H], F32, tag="ps")
                            for t in range(2):
                                for c in range(2):
                                    nc.tensor.matmul(
                                        ps[:, t, :],
                                        lhsT=wq[:, s, c, w,
                                                128 * t:128 * (t + 1)],
                                        rhs=xts[:, w, si, c, :],
                                        start=(c == 0),
                                        stop=(c == 1),
                                    )
                            dst = QTs if w == 0 else KTs
                            if (s + w) % 2 == 0:
                                nc.scalar.mul(dst[:, :, s, :], ps, 0.25)
                            else:
                                nc.vector.tensor_scalar_mul(
                                    out=dst[:, :, s, :], in0=ps, scalar1=0.25)

                # ---- Phase B: logits, softmax, attn @ V, store ----
                E = big.tile([128, N_GROUPS, NK], BF16, tag="E")
                rs = big.tile([128, N_GROUPS], F32, tag="rs")

                def do_quad(gq):
                    lg = lg_ps.tile([128, 4, NK], F32, tag="lg")
                    for qi in range(4):
                        g = 4 * gq + qi
                        for c in range(2):
                            for j in range(4):
                                b = 64 * j + g
                                nc.tensor.matmul(
                                    lg[32 * j:32 * (j + 1), qi, :],
                                    lhsT=QTs[:, c, :, b],
                                    rhs=KTs[:, c, :, b],
                                    start=(c == 0),
                                    stop=(c == 1),
                                    tile_position=(0, 32 * j),
                                    skip_group_check=True,
                                )
                    # softmax without max-subtraction: |logits| <= ~2.5
                    nc.scalar.activation(
                        out=E[:, 4 * gq:4 * gq + 4, :].rearrange(
                            "p a b -> p (a b)"),
                        in_=lg.rearrange("p a b -> p (a b)"),
                        func=mybir.ActivationFunctionType.Exp,
                    )

                for chunk in range(8):   # 8 groups per chunk
                    g0 = 8 * chunk
                    do_quad(2 * chunk)
                    do_quad(2 * chunk + 1)
                    sm = smp.tile([128, 8], F32, tag="sm")
                    nc.vector.reduce_sum(
                        out=sm, in_=E[:, g0:g0 + 8, :],
                        axis=mybir.AxisListType.X,
                    )
                    nc.vector.reciprocal(out=rs[:, g0:g0 + 8], in_=sm)
                    OUTo = outp.tile([128, 8, O], BF16, tag="OUTo")
                    for gg in (0, 4):
                        # 4-group attn transpose: te4[32j+m, g', n]
                        te4 = etp.tile([128, 4, NK], BF16, tag="te4")
                        nc.vector.transpose(
                            out=te4.rearrange("p a b -> p (a b)"),
                            in_=E[:, g0 + gg:g0 + gg + 4, :].rearrange(
                                "p a b -> p (a b)"),
                        )
                        for g2 in range(4):
                            g = g0 + gg + g2
                            av = mm_ps.tile([128, O], F32, tag="ps")
                            for j in range(4):
                                nc.tensor.matmul(
                                    av[32 * j:32 * (j + 1), :],
                                    lhsT=te4[32 * j:32 * (j + 1), g2, :],
                                    rhs=V32Q[32 * j:32 * (j + 1), g, :],
                                    start=True, stop=True,
                                    tile_position=(32 * j, 32 * j),
                                    skip_group_check=True,
                                )
                            # psum -> sbuf with 1/softmax-sum row scaling
                            if g % 2 == 0:
                                nc.scalar.mul(OUTo[:, g - g0, :], av,
                                              rs[:, g:g + 1])
                            else:
                                nc.vector.tensor_scalar_mul(
                                    out=OUTo[:, g - g0, :], in0=av,
                                    scalar1=rs[:, g:g + 1])
                    nc.sync.dma_start(
                        out=out_r[half, :, g0:g0 + 8, :], in_=OUTo)
    return nc


def _prep_inputs(q, k, query_weight, key_weight, value_weight):
    bf = ml_dtypes.bfloat16
    q = np.asarray(q, dtype=np.float32).astype(bf)
    k = np.asarray(k, dtype=np.float32).astype(bf)
    # xH[ci, half, sg, p, qk, s, c, b] = {q,k}[2sg+s, 512ci+256h+b, 128c+p]
    def pack_x(x):
        t = x.reshape(16, 2, N_CORES, N_HALVES, B_H, 2, 128)
        return t.transpose(2, 3, 0, 6, 1, 5, 4)  # [ci,half,sg,p,s,c,b]
    tq = pack_x(q)
    tk = pack_x(k)
    xAll = np.ascontiguousarray(
        np.stack((tq, tk), axis=4))  # [ci,half,sg,p,qk,s,c,b]
    # wqk[p, slot, c, qk, a]
    ws = np.stack(
        (np.asarray(query_weight, np.float32),
         np.asarray(key_weight, np.float32)), axis=2)  # [n, d, qk, a]
    wqk = np.ascontiguousarray(
        ws.reshape(NQ, 2, 128, 2, A).transpose(2, 0, 1, 3, 4).astype(bf))
    # wvH[sg, p, s, c, o]
    wv = np.ascontiguousarray(
        np.asarray(value_weight, np.float32)
        .reshape(16, 2, 2, 128, O).transpose(0, 3, 1, 2, 4).astype(bf))
    in_maps = []
    for i in range(N_CORES):
        in_maps.append({"xH": np.ascontiguousarray(xAll[i]),
                        "wqk": wqk, "wvH": wv})
    return in_maps


_NC_CACHE = {}


def _get_nc():
    if "nc" not in _NC_CACHE:
        nc = build_kernel()
        nc.finalize()
        _NC_CACHE["nc"] = nc
    return _NC_CACHE["nc"]


def kernel(q, k, query_weight, key_weight, value_weight, _trace=False):
    nc = _get_nc()
    in_maps = _prep_inputs(q, k, query_weight, key_weight, value_weight)
    res = run_bass_kernel_spmd(nc, in_maps, core_ids=list(range(N_CORES)),
                               trace=_trace)
    outs = []
    for i in range(N_CORES):
        o = res.results[i]["out"]  # [half, j, n, g, o]
        outs.append(o.transpose(2, 0, 1, 3, 4).reshape(NQ, BS_CORE, O))
    full = np.concatenate(outs, axis=1).astype(np.float32)
    if _trace:
        return full, res
    return full
